# revision 5
# baseline (speedup 1.0000x reference)
"""Trainium2 Bass kernel for DiceLoss (hard-argmax dice, ignore background, mean).

Problem (hardcoded shapes):
  y_true: [16, 512, 512] int32 in [0, 8)
  y_pred: [16, 8, 512, 512] float32
  out   : scalar float32 = mean over classes 1..7 of
          (2*tp + eps) / (2*tp + fp + fn + eps)
  with pred_cls = argmax_c y_pred, one-hot tp/fp/fn sums over all pixels.

Strategy (8 NeuronCores, data-parallel over batch; 2 images per core):
  - Each channel plane is one [128, 2048] tile. y_pred is loaded via SWDGE
    cast-DMA (f32 in HBM -> bf16 in SBUF): HBM read traffic is unchanged but
    every on-chip elementwise op runs in DVE 16-bit perf modes and no
    convert instructions are needed.
  - DVE (all bf16, no accum_out so the 2x/4x perf-mode uops stay eligible):
      * 7-op pairwise max tree -> m = max over channels      (2x_1P)
      * pred_c = (ch_c == m) via tensor_tensor is_equal      (2x_1P)
      * gt_c   = (tf == c) via tensor_single_scalar is_equal (4x_2P),
        written strided into a [128, 16, 130] block layout whose col 128
        holds a persistent ones column (memset once).
  - ScalarE: int32->bf16 label convert; per class a copy-with-accum_out of
    gt_c that yields the per-partition gt counts; PSUM evacuation.
  - TensorE: per class-subtile one matmul with lhsT = pred subtile and
    rhs = [gt subtile | ones] (129 cols) accumulated over subtiles+images:
    diag gives tp, column 128 gives pred counts. Host reads trace + sums.
  - Host: combines the 8 cores' exact-integer f32 partials and forms the
    dice mean in float32, matching the reference arithmetic.
"""

import numpy as np

EPS = 1e-05

# Problem geometry (hardcoded per the harness contract).
N_CORES = 8
NB = 2            # batch images per core
C = 8             # classes
P = 128           # SBUF partitions
FD = 2048         # free-dim elements per channel plane (512*512 = 128*2048)
NSUB = FD // 128  # 128-wide subtiles per plane for the PE matmuls
BLK = 130         # gt block stride: 128 gt cols + ones col + 1 pad (4B align)

_CACHED_NC = None


def build_bass():
    """Build the Bass kernel (same NEFF for all 8 cores)."""
    from contextlib import ExitStack

    import concourse.bacc as bacc
    import concourse.tile as tile
    from concourse import mybir

    nc = bacc.Bacc(None, target_bir_lowering=False)

    yp = nc.dram_tensor("yp", [NB, C, P, FD], mybir.dt.float32, kind="ExternalInput")
    yt = nc.dram_tensor("yt", [NB, P, FD], mybir.dt.int32, kind="ExternalInput")
    # per class: [128, 129] PSUM accumulator (cross-products + pred colsum).
    mm_out = nc.dram_tensor("mm_out", [7, P, 129], mybir.dt.float32, kind="ExternalOutput")
    # per-(image, class) per-partition gt counts
    ga_out = nc.dram_tensor("ga_out", [P, NB * 7], mybir.dt.float32, kind="ExternalOutput")

    with tile.TileContext(nc) as tc, ExitStack() as ctx:
        chpool = ctx.enter_context(tc.tile_pool(name="ch", bufs=2))
        tpool = ctx.enter_context(tc.tile_pool(name="tt", bufs=2))
        mpool = ctx.enter_context(tc.tile_pool(name="mx", bufs=2))
        mtmp = ctx.enter_context(tc.tile_pool(name="mtmp", bufs=6))
        predp = ctx.enter_context(tc.tile_pool(name="pred", bufs=3))
        scrp = ctx.enter_context(tc.tile_pool(name="scr", bufs=2))
        accp = ctx.enter_context(tc.tile_pool(name="acc", bufs=1))
        psump = ctx.enter_context(tc.tile_pool(name="psum", bufs=1, space="PSUM"))

        ga_acc = accp.tile([P, NB * 7], mybir.dt.float32, name="ga_acc")
        # fixed per-class gt tiles in block layout [128, 16, 130]:
        # cols 0:128 = gt mask (rewritten per image), col 128 = ones.
        gts = [
            accp.tile([P, NSUB, BLK], mybir.dt.bfloat16, name=f"gt{c}")
            for c in range(1, C)
        ]
        psums = [
            psump.tile([P, 129], mybir.dt.float32, name=f"ps{c}", tag=f"ps{c}")
            for c in range(1, C)
        ]

        for g in gts:
            nc.vector.memset(g[:, :, 128:129], 1.0)

        for n in range(NB):
            # SWDGE cast-DMA (f32 HBM -> bf16 SBUF), batched 4 channels per
            # transfer to keep the Q7 descriptor-generation cost low.
            chA = chpool.tile([P, 4, FD], mybir.dt.bfloat16, name="chA", tag="chA")
            nc.gpsimd.dma_start(out=chA, in_=yp[n, 0:4].rearrange("c p f -> p c f"))
            chB = chpool.tile([P, 4, FD], mybir.dt.bfloat16, name="chB", tag="chB")
            nc.gpsimd.dma_start(out=chB, in_=yp[n, 4:8].rearrange("c p f -> p c f"))
            ch = [chA[:, c, :] for c in range(4)] + [chB[:, c, :] for c in range(4)]
            ti = tpool.tile([P, NSUB, 128], mybir.dt.int32, name="ti", tag="ti")
            nc.sync.dma_start(out=ti, in_=yt[n])
            # labels to bf16 (exact for 0..7) on the otherwise idle ScalarE
            tf = tpool.tile([P, NSUB, 128], mybir.dt.bfloat16, name="tf", tag="tf")
            nc.scalar.copy(out=tf, in_=ti)

            # ---- max tree (DVE, bf16 tensor-tensor 2x mode) ----
            m01 = mtmp.tile([P, FD], mybir.dt.bfloat16, name="m01", tag="mt")
            nc.vector.tensor_max(m01, ch[0], ch[1])
            m23 = mtmp.tile([P, FD], mybir.dt.bfloat16, name="m23", tag="mt")
            nc.vector.tensor_max(m23, ch[2], ch[3])
            m45 = mtmp.tile([P, FD], mybir.dt.bfloat16, name="m45", tag="mt")
            nc.vector.tensor_max(m45, ch[4], ch[5])
            m67 = mtmp.tile([P, FD], mybir.dt.bfloat16, name="m67", tag="mt")
            nc.vector.tensor_max(m67, ch[6], ch[7])
            m0123 = mtmp.tile([P, FD], mybir.dt.bfloat16, name="m0123", tag="mt")
            nc.vector.tensor_max(m0123, m01, m23)
            m4567 = mtmp.tile([P, FD], mybir.dt.bfloat16, name="m4567", tag="mt")
            nc.vector.tensor_max(m4567, m45, m67)
            m = mpool.tile([P, FD], mybir.dt.bfloat16, name="m", tag="m")
            nc.vector.tensor_max(m, m0123, m4567)

            # ---- per-class masks + PE tp/pred-count ----
            for c in range(1, C):
                # gt first so the ScalarE count-copy can start while DVE
                # computes the pred mask.
                g = gts[c - 1]
                gv = g[:, :, 0:128]  # strided [128, 16, 128] view
                nc.vector.tensor_single_scalar(
                    out=gv, in_=tf, scalar=float(c), op=mybir.AluOpType.is_equal
                )
                pred = predp.tile([P, FD], mybir.dt.bfloat16, name=f"pred{c}", tag="pred")
                nc.vector.tensor_tensor(
                    out=pred, in0=ch[c], in1=m, op=mybir.AluOpType.is_equal
                )
                # gt count on ScalarE: copy with fused per-partition accum
                col = n * 7 + (c - 1)
                scr = scrp.tile([P, NSUB, 128], mybir.dt.bfloat16, name="scr", tag="scr")
                nc.scalar.activation(
                    out=scr,
                    in_=gv,
                    func=mybir.ActivationFunctionType.Copy,
                    accum_out=ga_acc[:, col : col + 1],
                )
                for s in range(NSUB):
                    nc.tensor.matmul(
                        psums[c - 1][:, :],
                        lhsT=pred[:, s * 128 : (s + 1) * 128],
                        rhs=g[:, s, 0:129],
                        start=(n == 0 and s == 0),
                        stop=(n == NB - 1 and s == NSUB - 1),
                    )

        for c in range(7):
            pt = accp.tile([P, 129], mybir.dt.float32, name=f"pt{c}", tag=f"pt{c}")
            nc.vector.tensor_copy(out=pt, in_=psums[c])
            nc.sync.dma_start(out=mm_out[c], in_=pt)
        nc.sync.dma_start(out=ga_out[:], in_=ga_acc)

    nc.finalize()
    return nc


def _get_bass():
    global _CACHED_NC
    if _CACHED_NC is None:
        _CACHED_NC = build_bass()
    return _CACHED_NC


def make_in_maps(y_true, y_pred):
    yp = np.ascontiguousarray(np.asarray(y_pred, dtype=np.float32))
    yt = np.ascontiguousarray(np.asarray(y_true, dtype=np.int32))
    in_maps = []
    for i in range(N_CORES):
        yps = np.ascontiguousarray(yp[NB * i : NB * (i + 1)]).reshape(NB, C, P, FD)
        yts = np.ascontiguousarray(yt[NB * i : NB * (i + 1)]).reshape(NB, P, FD)
        in_maps.append({"yp": yps, "yt": yts})
    return in_maps


def epilogue(results):
    """Combine the 8 cores' partial sums into the final dice mean (float32,
    mirroring the reference arithmetic)."""
    tp = np.zeros(7, dtype=np.float64)
    pred_cnt = np.zeros(7, dtype=np.float64)
    gt_cnt = np.zeros(7, dtype=np.float64)
    for r in results:
        mm = np.asarray(r["mm_out"], dtype=np.float64)  # [7, P, 129]
        tp += np.trace(mm[:, :, :128], axis1=1, axis2=2)
        pred_cnt += mm[:, :, 128].sum(axis=1)
        ga = np.asarray(r["ga_out"], dtype=np.float64).reshape(P, NB, 7)
        gt_cnt += ga.sum(axis=(0, 1))

    tp32 = tp.astype(np.float32)
    fp32_ = (pred_cnt - tp).astype(np.float32)
    fn32 = (gt_cnt - tp).astype(np.float32)
    eps = np.float32(EPS)
    two = np.float32(2.0)
    dice = (two * tp32 + eps) / (two * tp32 + fp32_ + fn32 + eps)
    return np.asarray(np.mean(dice, dtype=np.float32), dtype=np.float32)


def kernel(**inputs):
    from concourse.bass_utils import run_bass_kernel_spmd

    nc = _get_bass()
    in_maps = make_in_maps(inputs["y_true"], inputs["y_pred"])
    res = run_bass_kernel_spmd(nc, in_maps, core_ids=list(range(N_CORES)))
    return epilogue(res.results)


if __name__ == "__main__":
    # smoke test with random data
    rng = np.random.default_rng(0)
    y_true = rng.integers(0, C, size=(16, 512, 512)).astype(np.int32)
    y_pred = rng.standard_normal((16, C, 512, 512)).astype(np.float32)
    out = kernel(y_true=y_true, y_pred=y_pred)
    print("kernel output:", out)


# revision 8
# speedup vs baseline: 1.0993x; 1.0993x over previous
"""Trainium2 Bass kernel for DiceLoss (hard-argmax dice, ignore background, mean).

Problem (hardcoded shapes):
  y_true: [16, 512, 512] int32 in [0, 8)
  y_pred: [16, 8, 512, 512] float32
  out   : scalar float32 = mean over classes 1..7 of
          (2*tp + eps) / (2*tp + fp + fn + eps)
  with pred_cls = argmax_c y_pred, one-hot tp/fp/fn sums over all pixels.

Strategy (8 NeuronCores, data-parallel over batch; 2 images per core):
  - Each channel plane is one [128, 2048] tile. y_pred is loaded via SWDGE
    cast-DMA (f32 in HBM -> bf16 in SBUF): HBM read traffic is unchanged but
    every on-chip elementwise op runs in DVE 16-bit perf modes and no
    convert instructions are needed.
  - DVE (all bf16, no accum_out so the 2x/4x perf-mode uops stay eligible):
      * 7-op pairwise max tree -> m = max over channels      (2x_1P)
      * pred_c = (ch_c == m) via tensor_tensor is_equal      (2x_1P)
      * gt_c   = (tf == c) via tensor_single_scalar is_equal (4x_2P),
        written strided into a [128, 16, 130] block layout whose col 128
        holds a persistent ones column (memset once).
  - ScalarE: int32->bf16 label convert; per class a copy-with-accum_out of
    gt_c that yields the per-partition gt counts; PSUM evacuation.
  - TensorE: per class-subtile one matmul with lhsT = pred subtile and
    rhs = [gt subtile | ones] (129 cols) accumulated over subtiles+images:
    diag gives tp, column 128 gives pred counts. Host reads trace + sums.
  - Host: combines the 8 cores' exact-integer f32 partials and forms the
    dice mean in float32, matching the reference arithmetic.
"""

import numpy as np

EPS = 1e-05

# Problem geometry (hardcoded per the harness contract).
N_CORES = 8
NB = 2            # batch images per core
C = 8             # classes
P = 128           # SBUF partitions
FD = 2048         # free-dim elements per channel plane (512*512 = 128*2048)
NSUB = FD // 128  # 128-wide subtiles per plane for the PE matmuls
BLK = 130         # gt block stride: 128 gt cols + ones col + 1 pad (4B align)

_CACHED_NC = None


def build_bass():
    """Build the Bass kernel (same NEFF for all 8 cores)."""
    from contextlib import ExitStack

    import concourse.bacc as bacc
    import concourse.tile as tile
    from concourse import mybir

    nc = bacc.Bacc(None, target_bir_lowering=False)

    yp = nc.dram_tensor("yp", [NB, C, P, FD], mybir.dt.float32, kind="ExternalInput")
    yt = nc.dram_tensor("yt", [NB, P, FD], mybir.dt.int32, kind="ExternalInput")
    # per class: [128, 129] PSUM accumulator (cross-products + pred colsum).
    mm_out = nc.dram_tensor("mm_out", [7, P, 129], mybir.dt.float32, kind="ExternalOutput")
    # per-(image, class) per-partition gt counts
    ga_out = nc.dram_tensor("ga_out", [P, NB * 7], mybir.dt.float32, kind="ExternalOutput")

    with tile.TileContext(nc) as tc, ExitStack() as ctx:
        chpool = ctx.enter_context(tc.tile_pool(name="ch", bufs=1))
        tpool = ctx.enter_context(tc.tile_pool(name="tt", bufs=1))
        mpool = ctx.enter_context(tc.tile_pool(name="mx", bufs=2))
        mtmp = ctx.enter_context(tc.tile_pool(name="mtmp", bufs=6))
        predp = ctx.enter_context(tc.tile_pool(name="pred", bufs=4))
        scrp = ctx.enter_context(tc.tile_pool(name="scr", bufs=2))
        accp = ctx.enter_context(tc.tile_pool(name="acc", bufs=1))
        psump = ctx.enter_context(tc.tile_pool(name="psum", bufs=1, space="PSUM"))

        ga_acc = accp.tile([P, NB * 7], mybir.dt.float32, name="ga_acc")
        # fixed per-class gt tiles in block layout [128, 16, 130]:
        # cols 0:128 = gt mask (rewritten per image), col 128 = ones,
        # col 129 = zero pad (so a flat [128, 2080] read sums cleanly).
        gts = [
            accp.tile([P, NSUB, BLK], mybir.dt.bfloat16, name=f"gt{c}")
            for c in range(1, C)
        ]
        psums = [
            psump.tile([P, 129], mybir.dt.float32, name=f"ps{c}", tag=f"ps{c}")
            for c in range(1, C)
        ]

        for g in gts:
            nc.vector.memset(g[:, :, 128:129], 1.0)
            nc.vector.memset(g[:, :, 129:130], 0.0)

        # ---- all loads up front: gpsimd queue delivers the casts FIFO in
        # exactly this order; labels ride the concurrent HWDGE queue. ----
        ch = {}
        tf = {}
        for n in range(NB):
            for c in range(C):
                tl = chpool.tile([P, FD], mybir.dt.bfloat16, name=f"ch{c}", tag=f"n{n}ch{c}")
                # SWDGE cast-DMA: f32 HBM -> bf16 SBUF
                nc.gpsimd.dma_start(out=tl, in_=yp[n, c])
                ch[n, c] = tl
            ti = tpool.tile([P, FD], mybir.dt.int32, name="ti", tag=f"ti{n}")
            nc.sync.dma_start(out=ti, in_=yt[n])
            # labels to bf16 (exact for 0..7) on ScalarE; flat 2D keeps its
            # fast mode
            tfn = tpool.tile([P, FD], mybir.dt.bfloat16, name="tf", tag=f"tf{n}")
            nc.scalar.copy(out=tfn, in_=ti)
            tf[n] = tfn

        def emit_gt(n, c):
            """gt mask (DVE 4x) + gt count (ScalarE flat copy w/ accum)."""
            g = gts[c - 1]
            gv = g[:, :, 0:128]  # strided [128, 16, 128] view
            tf3 = tf[n].rearrange("p (s f) -> p s f", s=NSUB)
            nc.vector.tensor_single_scalar(
                out=gv, in_=tf3, scalar=float(c), op=mybir.AluOpType.is_equal
            )
            col = n * 7 + (c - 1)
            scr = scrp.tile([P, NSUB * BLK], mybir.dt.bfloat16, name="scr", tag="scr")
            # flat contiguous read (incl. ones + zero pad; host subtracts
            # the constant 16*128 per slot) keeps ScalarE in its fast mode
            nc.scalar.activation(
                out=scr,
                in_=g.rearrange("p s f -> p (s f)"),
                func=mybir.ActivationFunctionType.Copy,
                accum_out=ga_acc[:, col : col + 1],
            )

        def emit_pred_mm(n, c):
            pred = predp.tile([P, FD], mybir.dt.bfloat16, name=f"pred{c}", tag="pred")
            nc.vector.tensor_tensor(
                out=pred, in0=ch[n, c], in1=ms[n], op=mybir.AluOpType.is_equal
            )
            g = gts[c - 1]
            for s in range(NSUB):
                nc.tensor.matmul(
                    psums[c - 1][:, :],
                    lhsT=pred[:, s * 128 : (s + 1) * 128],
                    rhs=g[:, s, 0:129],
                    start=(n == 0 and s == 0),
                    stop=(n == NB - 1 and s == NSUB - 1),
                )

        # ---- DVE program, ordered to match SWDGE arrival times ----
        ms = {}
        for n in range(NB):
            # early tree half (ch0-3) first
            m01 = mtmp.tile([P, FD], mybir.dt.bfloat16, name="m01", tag="mt")
            nc.vector.tensor_max(m01, ch[n, 0], ch[n, 1])
            m23 = mtmp.tile([P, FD], mybir.dt.bfloat16, name="m23", tag="mt")
            nc.vector.tensor_max(m23, ch[n, 2], ch[n, 3])
            m0123 = mtmp.tile([P, FD], mybir.dt.bfloat16, name="m0123", tag="mt")
            nc.vector.tensor_max(m0123, m01, m23)
            if n == 0:
                # fill the ch4-7 arrival gap with the gt masks (need tf only)
                for c in range(1, C):
                    emit_gt(n, c)
            # late tree half: only these three ops trail the last channel
            m45 = mtmp.tile([P, FD], mybir.dt.bfloat16, name="m45", tag="mt")
            nc.vector.tensor_max(m45, ch[n, 4], ch[n, 5])
            m67 = mtmp.tile([P, FD], mybir.dt.bfloat16, name="m67", tag="mt")
            nc.vector.tensor_max(m67, ch[n, 6], ch[n, 7])
            m4567 = mtmp.tile([P, FD], mybir.dt.bfloat16, name="m4567", tag="mt")
            nc.vector.tensor_max(m4567, m45, m67)
            m = mpool.tile([P, FD], mybir.dt.bfloat16, name="m", tag="m")
            nc.vector.tensor_max(m, m0123, m4567)
            ms[n] = m

            if n == 0:
                for c in range(1, C):
                    emit_pred_mm(n, c)
                # image 1 gt masks overlap image 0's pred/MM phase
                for c in range(1, C):
                    emit_gt(1, c)
            else:
                for c in range(1, C):
                    emit_pred_mm(n, c)

        for c in range(7):
            pt = accp.tile([P, 129], mybir.dt.float32, name=f"pt{c}", tag=f"pt{c}")
            nc.scalar.copy(out=pt, in_=psums[c])
            nc.sync.dma_start(out=mm_out[c], in_=pt)
        nc.sync.dma_start(out=ga_out[:], in_=ga_acc)

    nc.finalize()
    return nc


def _get_bass():
    global _CACHED_NC
    if _CACHED_NC is None:
        _CACHED_NC = build_bass()
    return _CACHED_NC


def make_in_maps(y_true, y_pred):
    yp = np.ascontiguousarray(np.asarray(y_pred, dtype=np.float32))
    yt = np.ascontiguousarray(np.asarray(y_true, dtype=np.int32))
    in_maps = []
    for i in range(N_CORES):
        yps = np.ascontiguousarray(yp[NB * i : NB * (i + 1)]).reshape(NB, C, P, FD)
        yts = np.ascontiguousarray(yt[NB * i : NB * (i + 1)]).reshape(NB, P, FD)
        in_maps.append({"yp": yps, "yt": yts})
    return in_maps


def epilogue(results):
    """Combine the 8 cores' partial sums into the final dice mean (float32,
    mirroring the reference arithmetic)."""
    tp = np.zeros(7, dtype=np.float64)
    pred_cnt = np.zeros(7, dtype=np.float64)
    gt_cnt = np.zeros(7, dtype=np.float64)
    for r in results:
        mm = np.asarray(r["mm_out"], dtype=np.float64)  # [7, P, 129]
        tp += np.trace(mm[:, :, :128], axis1=1, axis2=2)
        pred_cnt += mm[:, :, 128].sum(axis=1)
        ga = np.asarray(r["ga_out"], dtype=np.float64).reshape(P, NB, 7)
        # each slot's flat accum includes the ones column: 16 per partition
        gt_cnt += ga.sum(axis=(0, 1)) - NB * (16 * P)

    tp32 = tp.astype(np.float32)
    fp32_ = (pred_cnt - tp).astype(np.float32)
    fn32 = (gt_cnt - tp).astype(np.float32)
    eps = np.float32(EPS)
    two = np.float32(2.0)
    dice = (two * tp32 + eps) / (two * tp32 + fp32_ + fn32 + eps)
    return np.asarray(np.mean(dice, dtype=np.float32), dtype=np.float32)


def kernel(**inputs):
    from concourse.bass_utils import run_bass_kernel_spmd

    nc = _get_bass()
    in_maps = make_in_maps(inputs["y_true"], inputs["y_pred"])
    res = run_bass_kernel_spmd(nc, in_maps, core_ids=list(range(N_CORES)))
    return epilogue(res.results)


if __name__ == "__main__":
    # smoke test with random data
    rng = np.random.default_rng(0)
    y_true = rng.integers(0, C, size=(16, 512, 512)).astype(np.int32)
    y_pred = rng.standard_normal((16, C, 512, 512)).astype(np.float32)
    out = kernel(y_true=y_true, y_pred=y_pred)
    print("kernel output:", out)
